# revision 13
# baseline (speedup 1.0000x reference)
"""Trainium2 Bass kernel for PhysicsPriorGenerator (histogram soft-binning).

Computes, for each batch row p[b] = [theta_a, d, theta_p]:
    mu[b,a]   = d / cos((theta_p + ANGLES[a]) * pi/180)
    P[b,d,a]  = exp(-(DEPTHS[d] - mu[b,a])^2 / (2*sigma^2))  masked to
                0 where not (0 < mu < 50), then column-normalized over d.
Returns (P_d, P_c) for inputs p and p_calib, each [256, 512, 256] f32.

Strategy (8 NeuronCores, batch-sharded):
  * Each core gets 32 rows of p + 32 rows of p_calib -> 64 local rows,
    processed as 32 "pairs" (2 rows per tile; free dim = 2*256 = 512).
  * (d - mu)^2 is produced directly by the PE as a K=3 matmul:
        [depths^2; depths; 1]^T @ [1; -2*mu'; mu'^2 + BIG*(1-valid)]
    (validity folded into the rhs: invalid columns get +1e4 so exp
    underflows to exactly 0).
  * ACT: exp(-2*x) PSUM->SBUF.  Column sums via PE matmul with all-ones
    [128,128] weights (result is already broadcast to all partitions),
    seeded with eps=1e-20 so invalid (all-zero) columns stay finite.
  * DVE: reciprocal of the summed tile, then P * recip multiplies.
    One of four multiplies per pair goes to GPSIMD (otherwise idle).
  * Output written as one 1 MiB DMA per pair, fully contiguous in DRAM.
"""

import sys

sys.path.insert(0, "/opt/trn_rl_repo")

from contextlib import ExitStack

import numpy as np

import concourse.bass as bass
import concourse.tile as tile
import concourse.bass_utils as bass_utils
from concourse import bacc, mybir

F32 = mybir.dt.float32
AF = mybir.ActivationFunctionType
ALU = mybir.AluOpType

B = 256  # full batch per tensor
D = 512
A = 256
N_CORES = 8
BLOC = B // N_CORES  # 32 rows of p (+32 of p_calib) per core
ROWS = 2 * BLOC  # 64 local batch rows per core
PAIRS = ROWS // 2  # 32 pairs, 2 rows each
W = 2 * A  # 512, free width of one pair tile
DBLK = D // 128  # 4 depth blocks of 128 partitions

MAX_DEPTH = 50.0
SIGMA = 0.5
EXP_SCALE = -1.0 / (2.0 * SIGMA * SIGMA)  # -2.0
DEG2RAD = float(np.float32(np.pi / 180.0))
BIG = 1.0e4  # pushed into sq for invalid columns -> exp == +0.0
EPS = 1.0e-20  # colsum seed so reciprocal of empty columns is finite

import os

ROW_TILED = os.environ.get("K_ROW_TILED", "0") == "1"

_CACHE = {}


def _constants():
    import jax.numpy as jnp

    angles = np.asarray(jnp.linspace(-30.0, 30.0, A), dtype=np.float32)  # degrees
    depths = np.asarray(jnp.linspace(0.0, MAX_DEPTH, D), dtype=np.float32)
    angles_bcast = np.ascontiguousarray(np.broadcast_to(angles, (ROWS, A)))
    if ROW_TILED:
        # PE row-group i (SBUF partitions 32i..32i+2) holds
        # [depths^2; depths; 1] for depth block i.
        lhsT = np.zeros((128, 128), np.float32)
        for i in range(DBLK):
            blk = depths[i * 128 : (i + 1) * 128]
            lhsT[32 * i + 0] = blk * blk
            lhsT[32 * i + 1] = blk
            lhsT[32 * i + 2] = 1.0
    else:
        lhsT = np.zeros((3, D), np.float32)
        lhsT[0] = depths * depths
        lhsT[1] = depths
        lhsT[2] = 1.0
    return angles_bcast, np.ascontiguousarray(lhsT)


def _build():
    nc = bacc.Bacc("TRN2", target_bir_lowering=False, debug=False, num_devices=N_CORES)

    p_loc_d = nc.dram_tensor("p_loc", (ROWS, 3), F32, kind="ExternalInput").ap()
    ang_d = nc.dram_tensor("angles", (ROWS, A), F32, kind="ExternalInput").ap()
    lhsT_shape = (128, 128) if ROW_TILED else (3, D)
    lhsT_d = nc.dram_tensor("lhsT", lhsT_shape, F32, kind="ExternalInput").ap()
    rones_d = nc.dram_tensor("rhs_ones", (1, ROWS * A), F32, kind="ExternalInput").ap()
    onesc_d = nc.dram_tensor(
        "ones_col", (128, 128), mybir.dt.float32r, kind="ExternalInput"
    ).ap()
    onesr_d = nc.dram_tensor(
        "ones_eps", (1, 128 + W), mybir.dt.float32r, kind="ExternalInput"
    ).ap()
    out_d = nc.dram_tensor("out", (ROWS, D, A), F32, kind="ExternalOutput").ap()

    with tile.TileContext(nc) as tc, ExitStack() as ctx:
        const = ctx.enter_context(tc.tile_pool(name="const", bufs=1))
        prep = ctx.enter_context(tc.tile_pool(name="prep", bufs=1))
        pun_pool = ctx.enter_context(tc.tile_pool(name="pun", bufs=3))
        pout_pool = ctx.enter_context(tc.tile_pool(name="pout", bufs=4))
        rec_pool = ctx.enter_context(tc.tile_pool(name="rec", bufs=3))
        sq_pool = ctx.enter_context(tc.tile_pool(name="sq", bufs=5, space="PSUM"))
        fb_pool = ctx.enter_context(tc.tile_pool(name="fb", bufs=2, space="PSUM"))

        # ---- constants / inputs ----
        p_sb = const.tile([ROWS, 3], F32)
        ang_sb = const.tile([ROWS, A], F32)
        lhsT_sb = const.tile(list(lhsT_shape), F32)
        nc.sync.dma_start(p_sb, p_loc_d)
        nc.sync.dma_start(ang_sb, ang_d)
        nc.sync.dma_start(lhsT_sb, lhsT_d)

        F32R = mybir.dt.float32r
        ones_col = const.tile([128, 128], F32R)  # colsum weights (K=128, M=128)
        nc.sync.dma_start(ones_col, onesc_d)
        ones_eps = const.tile([1, 128 + W], F32R)
        nc.sync.dma_start(ones_eps, onesr_d)
        ones_row = ones_eps[:, 0:128]  # eps weights (K=1, M=128)
        eps_row = ones_eps[:, 128 : 128 + W]

        # ---- per-row prep: mu, masked rhs rows ----
        theta_bias = prep.tile([ROWS, 1], F32)
        # theta_p * pi/180 + pi/2   (pi/2 turns ACT Sin into cos)
        nc.vector.tensor_scalar(
            theta_bias, p_sb[:, 2:3], DEG2RAD, float(np.pi / 2), ALU.mult, ALU.add
        )
        cosang = prep.tile([ROWS, A], F32)
        nc.scalar.activation(
            cosang, ang_sb, AF.Sin, bias=theta_bias[:], scale=DEG2RAD
        )
        rcos = prep.tile([ROWS, A], F32)
        scratch = prep.tile([ROWS, A], F32)
        nc.vector.reciprocal_approx_accurate(rcos, cosang, scratch)
        mu = prep.tile([ROWS, A], F32)
        nc.vector.tensor_scalar_mul(mu, rcos, p_sb[:, 1:2])
        m2mu = prep.tile([ROWS, A], F32)
        nc.vector.tensor_scalar_mul(m2mu, mu, -2.0)
        musq = prep.tile([ROWS, A], F32)
        nc.vector.tensor_mul(musq, mu, mu)
        v_lt = prep.tile([ROWS, A], F32)
        nc.vector.tensor_scalar(v_lt, mu, float(MAX_DEPTH), None, ALU.is_lt)
        v_gt = prep.tile([ROWS, A], F32)
        nc.vector.tensor_scalar(v_gt, mu, 0.0, None, ALU.is_gt)
        vld = prep.tile([ROWS, A], F32)
        nc.vector.tensor_mul(vld, v_lt, v_gt)

        m2mu_m = prep.tile([ROWS, A], F32)
        nc.vector.tensor_mul(m2mu_m, m2mu, vld)
        musq_v = prep.tile([ROWS, A], F32)
        nc.vector.tensor_mul(musq_v, musq, vld)
        bigt = prep.tile([ROWS, A], F32)
        nc.vector.tensor_scalar(bigt, vld, -BIG, BIG, ALU.mult, ALU.add)
        musq_m = prep.tile([ROWS, A], F32)
        nc.vector.tensor_add(musq_m, musq_v, bigt)

        # ---- flatten rhs rows to [1, 16384] (replicated per row-group when
        # row-tiled); ones row comes from DRAM (memset on 1 partition is slow)
        if ROW_TILED:
            rhs_all = const.tile([128, ROWS * A], F32)
            bases = [32 * i for i in range(DBLK)]
        else:
            rhs_all = const.tile([3, ROWS * A], F32)
            bases = [0]
        for b0 in bases:
            nc.sync.dma_start(rhs_all[b0 : b0 + 1, :], rones_d)
            nc.sync.dma_start(rhs_all[b0 + 1 : b0 + 2, :], m2mu_m[:, :])
            nc.sync.dma_start(rhs_all[b0 + 2 : b0 + 3, :], musq_m[:, :])

        # ---- main loop over pairs ----
        for pl in range(PAIRS):
            sq_tiles = []
            for k in range(DBLK):
                sq_ps = sq_pool.tile([128, W], F32, tag="sq")
                if ROW_TILED:
                    nc.tensor.matmul(
                        sq_ps,
                        lhsT_sb[32 * k : 32 * k + 3, :],
                        rhs_all[32 * k : 32 * k + 3, pl * W : (pl + 1) * W],
                        start=True,
                        stop=True,
                        tile_position=(32 * k, 0),
                    )
                else:
                    nc.tensor.matmul(
                        sq_ps,
                        lhsT_sb[:, k * 128 : (k + 1) * 128],
                        rhs_all[:, pl * W : (pl + 1) * W],
                        start=True,
                        stop=True,
                    )
                sq_tiles.append(sq_ps)

            p_un = pun_pool.tile([128, DBLK * W], F32R)
            for k in range(DBLK):
                nc.scalar.activation(
                    p_un[:, k * W : (k + 1) * W], sq_tiles[k], AF.Exp, scale=EXP_SCALE
                )

            fb_ps = fb_pool.tile([128, W], F32)
            if ROW_TILED:
                nc.tensor.matmul(
                    fb_ps,
                    ones_row,
                    eps_row,
                    start=True,
                    stop=False,
                    tile_position=(0, 0),
                )
                for k in range(DBLK):
                    for g in range(4):
                        nc.tensor.matmul(
                            fb_ps,
                            ones_col[32 * g : 32 * g + 32, :],
                            p_un[32 * g : 32 * g + 32, k * W : (k + 1) * W],
                            start=False,
                            stop=(k == DBLK - 1 and g == 3),
                            tile_position=(32 * g, 0),
                        )
            else:
                nc.tensor.matmul(fb_ps, ones_row, eps_row, start=True, stop=False)
                for k in range(DBLK):
                    nc.tensor.matmul(
                        fb_ps,
                        ones_col,
                        p_un[:, k * W : (k + 1) * W],
                        start=False,
                        stop=(k == DBLK - 1),
                    )

            rec = rec_pool.tile([128, W], F32)
            nc.vector.reciprocal_approx_fast(rec, fb_ps)

            # p_out free layout is (b, dblk, a) so the store below is a
            # single 3-dim AP on both sides (src fully contiguous).
            p_out = pout_pool.tile([128, DBLK * W], F32)
            p_out_r = p_out.rearrange("p (b k a) -> p b k a", b=2, k=DBLK)
            rec_r = rec.rearrange("p (b a) -> p b a", b=2)
            for k in range(DBLK):
                eng = nc.gpsimd if k == 3 else nc.vector
                eng.tensor_mul(
                    p_out_r[:, :, k, :],
                    p_un[:, k * W : (k + 1) * W]
                    .bitcast(F32)
                    .rearrange("p (b a) -> p b a", b=2),
                    rec_r,
                )

            # one contiguous 1 MiB store: rows (2*pl, 2*pl+1) of out
            dst = bass.AP(
                tensor=out_d.tensor,
                offset=(2 * pl) * D * A,
                ap=[[A, 128], [128 * A, 2 * DBLK], [1, A]],
            )
            nc.sync.dma_start(dst, p_out)

    nc.compile()
    return nc


def _get_nc():
    if "nc" not in _CACHE:
        _CACHE["nc"] = _build()
    return _CACHE["nc"]


def _in_maps(p, p_calib):
    angles_bcast, lhsT = _constants()

    ones_col_np = np.ones((128, 128), np.float32)
    rhs_ones_np = np.ones((1, ROWS * A), np.float32)
    ones_eps_np = np.concatenate(
        [np.ones(128, np.float32), np.full(W, EPS, np.float32)]
    )[None, :]

    in_maps = []
    for c in range(N_CORES):
        p_loc = np.concatenate(
            [p[c * BLOC : (c + 1) * BLOC], p_calib[c * BLOC : (c + 1) * BLOC]], axis=0
        )
        in_maps.append(
            {
                "p_loc": np.ascontiguousarray(p_loc),
                "angles": angles_bcast,
                "lhsT": lhsT,
                "ones_col": ones_col_np,
                "ones_eps": np.ascontiguousarray(ones_eps_np),
                "rhs_ones": rhs_ones_np,
            }
        )
    return in_maps


def kernel(p, p_calib):
    p = np.ascontiguousarray(np.asarray(p, dtype=np.float32))
    p_calib = np.ascontiguousarray(np.asarray(p_calib, dtype=np.float32))
    assert p.shape == (B, 3) and p_calib.shape == (B, 3)

    nc = _get_nc()
    in_maps = _in_maps(p, p_calib)
    res = bass_utils.run_bass_kernel_spmd(nc, in_maps, core_ids=list(range(N_CORES)))

    P_d = np.empty((B, D, A), np.float32)
    P_c = np.empty((B, D, A), np.float32)
    for c in range(N_CORES):
        o = res.results[c]["out"]
        P_d[c * BLOC : (c + 1) * BLOC] = o[:BLOC]
        P_c[c * BLOC : (c + 1) * BLOC] = o[BLOC:]
    return (P_d, P_c)


# revision 14
# speedup vs baseline: 1.4333x; 1.4333x over previous
"""Trainium2 Bass kernel for PhysicsPriorGenerator (histogram soft-binning).

Computes, for each batch row p[b] = [theta_a, d, theta_p]:
    mu[b,a]   = d / cos((theta_p + ANGLES[a]) * pi/180)
    P[b,d,a]  = exp(-(DEPTHS[d] - mu[b,a])^2 / (2*sigma^2))  masked to
                0 where not (0 < mu < 50), then column-normalized over d.
Returns (P_d, P_c) for inputs p and p_calib, each [256, 512, 256] f32.

Strategy (8 NeuronCores, batch-sharded; 64 rows/core as 32 row-pairs):
  * (d - mu)^2 = d^2 - 2 d mu + mu^2 is produced directly on the PE as a
    single-pass bf16 K=16 matmul: d^2 and mu^2 are split into 4 bf16
    terms each, the cross term into 8 bf16 x bf16 products (exact in the
    f32 PSUM accumulation); rows are ordered so the running sum stays
    near the true (small) value.  Validity is folded into the rhs:
    invalid columns get mu^2 -> +1e4 so exp underflows to exactly 0.
  * ACT: exp(-2*x) PSUM->SBUF (float32r so the colsum matmul may
    consume it).  Column sums via PE float32r matmul with all-ones
    [128,128] weights - the result is already broadcast to all
    partitions - seeded with eps=1e-20 so empty columns stay finite.
  * DVE: fast reciprocal of the summed tile, then P * recip multiplies
    (one of four per pair on GPSIMD, otherwise idle).
  * Output written as one 1 MiB DMA per pair, fully contiguous in DRAM.
"""

import os
import sys

sys.path.insert(0, "/opt/trn_rl_repo")

from contextlib import ExitStack

import ml_dtypes
import numpy as np

import concourse.bass as bass
import concourse.tile as tile
import concourse.bass_utils as bass_utils
from concourse import bacc, mybir

F32 = mybir.dt.float32
F32R = mybir.dt.float32r
BF16 = mybir.dt.bfloat16
AF = mybir.ActivationFunctionType
ALU = mybir.AluOpType

B = 256  # full batch per tensor
D = 512
A = 256
N_CORES = 8
BLOC = B // N_CORES  # 32 rows of p (+32 of p_calib) per core
ROWS = 2 * BLOC  # 64 local batch rows per core
PAIRS = ROWS // 2  # 32 pairs, 2 rows each
W = 2 * A  # 512, free width of one pair tile
DBLK = D // 128  # 4 depth blocks of 128 partitions
K16 = 16  # contraction depth of the bf16-split diff^2 matmul

MAX_DEPTH = 50.0
SIGMA = 0.5
EXP_SCALE = -1.0 / (2.0 * SIGMA * SIGMA)  # -2.0
DEG2RAD = float(np.float32(np.pi / 180.0))
BIG = 1.0e4  # pushed into sq for invalid columns -> exp == +0.0
EPS = 1.0e-20  # colsum seed so reciprocal of empty columns is finite

ROW_TILED = os.environ.get("K_ROW_TILED", "0") == "1"

# Term order (magnitude tiers): lhsT row i pairs with rhs row i.
#   q_j = bf16 split j of depths^2   (rhs = ones)
#   d_j = bf16 split j of depths     (rhs = m_k = split k of -2*mu)
#   s_j = bf16 split j of mu^2+mask  (lhsT = ones)
_LHS_ROWS = ["q1", "d1", "1", "q2", "d1", "d2", "1", "q3", "d1", "d2", "d3", "1",
             "q4", "d2", "d3", "1"]
_RHS_ROWS = ["1", "m1", "s1", "1", "m2", "m1", "s2", "1", "m3", "m2", "m1", "s3",
             "1", "m3", "m2", "s4"]

_CACHE = {}


def _bf16_splits(x, n):
    out = []
    r = np.asarray(x, np.float32).copy()
    for _ in range(n):
        h = r.astype(ml_dtypes.bfloat16)
        out.append(h)
        r = r - h.astype(np.float32)
    return out


def _constants():
    import jax.numpy as jnp

    angles = np.asarray(jnp.linspace(-30.0, 30.0, A), dtype=np.float32)  # degrees
    depths = np.asarray(jnp.linspace(0.0, MAX_DEPTH, D), dtype=np.float32)
    angles_bcast = np.ascontiguousarray(np.broadcast_to(angles, (ROWS, A)))

    d1, d2, d3 = _bf16_splits(depths, 3)
    q1, q2, q3, q4 = _bf16_splits(depths * depths, 4)
    ones = np.ones(D, ml_dtypes.bfloat16)
    terms = {"d1": d1, "d2": d2, "d3": d3, "q1": q1, "q2": q2, "q3": q3, "q4": q4,
             "1": ones}

    if ROW_TILED:
        lhsT = np.zeros((128, 128), ml_dtypes.bfloat16)
        for i in range(DBLK):
            for j, nm in enumerate(_LHS_ROWS):
                lhsT[32 * i + j] = terms[nm][i * 128 : (i + 1) * 128]
    else:
        lhsT = np.zeros((K16, D), ml_dtypes.bfloat16)
        for j, nm in enumerate(_LHS_ROWS):
            lhsT[j] = terms[nm]
    return angles_bcast, np.ascontiguousarray(lhsT)


def _build():
    nc = bacc.Bacc("TRN2", target_bir_lowering=False, debug=False, num_devices=N_CORES)

    lhsT_shape = (128, 128) if ROW_TILED else (K16, D)
    p_loc_d = nc.dram_tensor("p_loc", (ROWS, 3), F32, kind="ExternalInput").ap()
    ang_d = nc.dram_tensor("angles", (ROWS, A), F32, kind="ExternalInput").ap()
    lhsT_d = nc.dram_tensor("lhsT", lhsT_shape, BF16, kind="ExternalInput").ap()
    rones_d = nc.dram_tensor("rhs_ones", (1, ROWS * A), BF16, kind="ExternalInput").ap()
    onesc_d = nc.dram_tensor(
        "ones_col", (128, 128), F32R, kind="ExternalInput"
    ).ap()
    onesr_d = nc.dram_tensor(
        "ones_eps", (1, 128 + W), F32R, kind="ExternalInput"
    ).ap()
    out_d = nc.dram_tensor("out", (ROWS, D, A), F32, kind="ExternalOutput").ap()

    with tile.TileContext(nc) as tc, ExitStack() as ctx:
        const = ctx.enter_context(tc.tile_pool(name="const", bufs=1))
        prep = ctx.enter_context(tc.tile_pool(name="prep", bufs=1))
        pun_pool = ctx.enter_context(tc.tile_pool(name="pun", bufs=3))
        pout_pool = ctx.enter_context(tc.tile_pool(name="pout", bufs=4))
        rec_pool = ctx.enter_context(tc.tile_pool(name="rec", bufs=3))
        sq_pool = ctx.enter_context(tc.tile_pool(name="sq", bufs=5, space="PSUM"))
        fb_pool = ctx.enter_context(tc.tile_pool(name="fb", bufs=2, space="PSUM"))

        # ---- constants / inputs ----
        p_sb = const.tile([ROWS, 3], F32)
        ang_sb = const.tile([ROWS, A], F32)
        lhsT_sb = const.tile(list(lhsT_shape), BF16)
        nc.sync.dma_start(p_sb, p_loc_d)
        nc.sync.dma_start(ang_sb, ang_d)
        nc.sync.dma_start(lhsT_sb, lhsT_d)

        ones_col = const.tile([128, 128], F32R)  # colsum weights (K=128, M=128)
        nc.sync.dma_start(ones_col, onesc_d)
        ones_eps = const.tile([1, 128 + W], F32R)
        nc.sync.dma_start(ones_eps, onesr_d)
        ones_row = ones_eps[:, 0:128]  # eps weights (K=1, M=128)
        eps_row = ones_eps[:, 128 : 128 + W]

        # ---- per-row prep: mu, masked rhs rows ----
        theta_bias = prep.tile([ROWS, 1], F32)
        # theta_p * pi/180 + pi/2   (pi/2 turns ACT Sin into cos)
        nc.vector.tensor_scalar(
            theta_bias, p_sb[:, 2:3], DEG2RAD, float(np.pi / 2), ALU.mult, ALU.add
        )
        cosang = prep.tile([ROWS, A], F32)
        nc.scalar.activation(cosang, ang_sb, AF.Sin, bias=theta_bias[:], scale=DEG2RAD)
        rcos = prep.tile([ROWS, A], F32)
        scratch = prep.tile([ROWS, A], F32)
        nc.vector.reciprocal_approx_accurate(rcos, cosang, scratch)
        mu = prep.tile([ROWS, A], F32)
        nc.vector.tensor_scalar_mul(mu, rcos, p_sb[:, 1:2])
        m2mu = prep.tile([ROWS, A], F32)
        nc.vector.tensor_scalar_mul(m2mu, mu, -2.0)
        musq = prep.tile([ROWS, A], F32)
        nc.vector.tensor_mul(musq, mu, mu)
        v_lt = prep.tile([ROWS, A], F32)
        nc.vector.tensor_scalar(v_lt, mu, float(MAX_DEPTH), None, ALU.is_lt)
        v_gt = prep.tile([ROWS, A], F32)
        nc.vector.tensor_scalar(v_gt, mu, 0.0, None, ALU.is_gt)
        vld = prep.tile([ROWS, A], F32)
        nc.vector.tensor_mul(vld, v_lt, v_gt)

        m2mu_m = prep.tile([ROWS, A], F32)
        nc.vector.tensor_mul(m2mu_m, m2mu, vld)
        musq_v = prep.tile([ROWS, A], F32)
        nc.vector.tensor_mul(musq_v, musq, vld)
        bigt = prep.tile([ROWS, A], F32)
        nc.vector.tensor_scalar(bigt, vld, -BIG, BIG, ALU.mult, ALU.add)
        musq_m = prep.tile([ROWS, A], F32)
        nc.vector.tensor_add(musq_m, musq_v, bigt)

        # ---- bf16 splits of -2*mu (3 terms) and mu^2+mask (4 terms) ----
        def dev_splits(src, n, tag):
            outs = []
            r = src
            for t in range(n):
                h = prep.tile([ROWS, A], BF16, tag=f"{tag}h{t}")
                nc.vector.tensor_copy(h, r)
                outs.append(h)
                if t < n - 1:
                    nr = prep.tile([ROWS, A], F32, tag=f"{tag}r{t}")
                    nc.vector.tensor_tensor(nr, r, h, ALU.subtract)
                    r = nr
            return outs

        m1, m2, m3 = dev_splits(m2mu_m, 3, "m")
        s1, s2, s3, s4 = dev_splits(musq_m, 4, "s")
        rhs_srcs = {"m1": m1, "m2": m2, "m3": m3, "s1": s1, "s2": s2, "s3": s3,
                    "s4": s4}

        # ---- flatten rhs rows to [1, 16384] bf16 ----
        if ROW_TILED:
            rhs_all = const.tile([128, ROWS * A], BF16)
            bases = [32 * i for i in range(DBLK)]
        else:
            rhs_all = const.tile([K16, ROWS * A], BF16)
            bases = [0]
        for b0 in bases:
            for j, nm in enumerate(_RHS_ROWS):
                row = rhs_all[b0 + j : b0 + j + 1, :]
                if nm == "1":
                    nc.sync.dma_start(row, rones_d)
                else:
                    nc.sync.dma_start(row, rhs_srcs[nm][:, :])

        # ---- main loop over pairs ----
        for pl in range(PAIRS):
            sq_tiles = []
            for k in range(DBLK):
                sq_ps = sq_pool.tile([128, W], F32, tag="sq")
                if ROW_TILED:
                    nc.tensor.matmul(
                        sq_ps,
                        lhsT_sb[32 * k : 32 * k + K16, :],
                        rhs_all[32 * k : 32 * k + K16, pl * W : (pl + 1) * W],
                        start=True,
                        stop=True,
                        tile_position=(32 * k, 0),
                    )
                else:
                    nc.tensor.matmul(
                        sq_ps,
                        lhsT_sb[:, k * 128 : (k + 1) * 128],
                        rhs_all[:, pl * W : (pl + 1) * W],
                        start=True,
                        stop=True,
                    )
                sq_tiles.append(sq_ps)

            p_un = pun_pool.tile([128, DBLK * W], F32R)
            for k in range(DBLK):
                nc.scalar.activation(
                    p_un[:, k * W : (k + 1) * W], sq_tiles[k], AF.Exp, scale=EXP_SCALE
                )

            fb_ps = fb_pool.tile([128, W], F32)
            if ROW_TILED:
                nc.tensor.matmul(
                    fb_ps, ones_row, eps_row, start=True, stop=False,
                    tile_position=(0, 0),
                )
                for k in range(DBLK):
                    for g in range(4):
                        nc.tensor.matmul(
                            fb_ps,
                            ones_col[32 * g : 32 * g + 32, :],
                            p_un[32 * g : 32 * g + 32, k * W : (k + 1) * W],
                            start=False,
                            stop=(k == DBLK - 1 and g == 3),
                            tile_position=(32 * g, 0),
                        )
            else:
                nc.tensor.matmul(fb_ps, ones_row, eps_row, start=True, stop=False)
                for k in range(DBLK):
                    nc.tensor.matmul(
                        fb_ps,
                        ones_col,
                        p_un[:, k * W : (k + 1) * W],
                        start=False,
                        stop=(k == DBLK - 1),
                    )

            rec = rec_pool.tile([128, W], F32)
            nc.vector.reciprocal_approx_fast(rec, fb_ps)

            # p_out free layout is (b, dblk, a) so the store below is a
            # single 3-dim AP on both sides (src fully contiguous).
            p_out = pout_pool.tile([128, DBLK * W], F32)
            p_out_r = p_out.rearrange("p (b k a) -> p b k a", b=2, k=DBLK)
            rec_r = rec.rearrange("p (b a) -> p b a", b=2)
            for k in range(DBLK):
                eng = nc.gpsimd if k == 3 else nc.vector
                eng.tensor_mul(
                    p_out_r[:, :, k, :],
                    p_un[:, k * W : (k + 1) * W]
                    .bitcast(F32)
                    .rearrange("p (b a) -> p b a", b=2),
                    rec_r,
                )

            # one contiguous 1 MiB store: rows (2*pl, 2*pl+1) of out
            dst = bass.AP(
                tensor=out_d.tensor,
                offset=(2 * pl) * D * A,
                ap=[[A, 128], [128 * A, 2 * DBLK], [1, A]],
            )
            nc.sync.dma_start(dst, p_out)

    nc.compile()
    return nc


def _get_nc():
    if "nc" not in _CACHE:
        _CACHE["nc"] = _build()
    return _CACHE["nc"]


def _in_maps(p, p_calib):
    angles_bcast, lhsT = _constants()

    ones_col_np = np.ones((128, 128), np.float32)
    ones_eps_np = np.concatenate(
        [np.ones(128, np.float32), np.full(W, EPS, np.float32)]
    )[None, :]
    rhs_ones_np = np.ones((1, ROWS * A), ml_dtypes.bfloat16)

    in_maps = []
    for c in range(N_CORES):
        p_loc = np.concatenate(
            [p[c * BLOC : (c + 1) * BLOC], p_calib[c * BLOC : (c + 1) * BLOC]], axis=0
        )
        in_maps.append(
            {
                "p_loc": np.ascontiguousarray(p_loc),
                "angles": angles_bcast,
                "lhsT": lhsT,
                "ones_col": ones_col_np,
                "ones_eps": np.ascontiguousarray(ones_eps_np),
                "rhs_ones": rhs_ones_np,
            }
        )
    return in_maps


def kernel(p, p_calib):
    p = np.ascontiguousarray(np.asarray(p, dtype=np.float32))
    p_calib = np.ascontiguousarray(np.asarray(p_calib, dtype=np.float32))
    assert p.shape == (B, 3) and p_calib.shape == (B, 3)

    nc = _get_nc()
    in_maps = _in_maps(p, p_calib)
    res = bass_utils.run_bass_kernel_spmd(nc, in_maps, core_ids=list(range(N_CORES)))

    P_d = np.empty((B, D, A), np.float32)
    P_c = np.empty((B, D, A), np.float32)
    for c in range(N_CORES):
        o = res.results[c]["out"]
        P_d[c * BLOC : (c + 1) * BLOC] = o[:BLOC]
        P_c[c * BLOC : (c + 1) * BLOC] = o[BLOC:]
    return (P_d, P_c)


# revision 15
# speedup vs baseline: 1.6927x; 1.1810x over previous
"""Trainium2 Bass kernel for PhysicsPriorGenerator (histogram soft-binning).

Computes, for each batch row p[b] = [theta_a, d, theta_p]:
    mu[b,a]   = d / cos((theta_p + ANGLES[a]) * pi/180)
    P[b,d,a]  = exp(-(DEPTHS[d] - mu[b,a])^2 / (2*sigma^2))  masked to
                0 where not (0 < mu < 50), then column-normalized over d.
Returns (P_d, P_c) for inputs p and p_calib, each [256, 512, 256] f32.

Strategy (8 NeuronCores, batch-sharded; 64 rows/core as 32 row-pairs):
  * (d - mu)^2 = d^2 - 2 d mu + mu^2 is produced directly on the PE as a
    single-pass bf16 K=16 matmul: d^2 and mu^2 are split into 4 bf16
    terms each, the cross term into 8 bf16 x bf16 products (exact in the
    f32 PSUM accumulation); rows are ordered so the running sum stays
    near the true (small) value.  Validity is folded into the rhs:
    invalid columns get mu^2 -> +1e4 so exp underflows to exactly 0.
  * ACT: exp(-2*x) PSUM->SBUF (float32r so the colsum matmul may
    consume it).  Column sums via PE float32r matmul with all-ones
    [128,128] weights - the result is already broadcast to all
    partitions - seeded with eps=1e-20 so empty columns stay finite.
  * DVE: fast reciprocal of the summed tile, then P * recip multiplies
    (one of four per pair on GPSIMD, otherwise idle).
  * Output written as one 1 MiB DMA per pair, fully contiguous in DRAM.
"""

import os
import sys

sys.path.insert(0, "/opt/trn_rl_repo")

from contextlib import ExitStack

import ml_dtypes
import numpy as np

import concourse.bass as bass
import concourse.tile as tile
import concourse.bass_utils as bass_utils
from concourse import bacc, mybir

F32 = mybir.dt.float32
F32R = mybir.dt.float32r
BF16 = mybir.dt.bfloat16
AF = mybir.ActivationFunctionType
ALU = mybir.AluOpType

B = 256  # full batch per tensor
D = 512
A = 256
N_CORES = 8
BLOC = B // N_CORES  # 32 rows of p (+32 of p_calib) per core
ROWS = 2 * BLOC  # 64 local batch rows per core
PAIRS = ROWS // 2  # 32 pairs, 2 rows each
W = 2 * A  # 512, free width of one pair tile
DBLK = D // 128  # 4 depth blocks of 128 partitions
K16 = 16  # contraction depth of the bf16-split diff^2 matmul

MAX_DEPTH = 50.0
SIGMA = 0.5
EXP_SCALE = -1.0 / (2.0 * SIGMA * SIGMA)  # -2.0
DEG2RAD = float(np.float32(np.pi / 180.0))
BIG = 1.0e4  # pushed into sq for invalid columns -> exp == +0.0
EPS = 1.0e-20  # colsum seed so reciprocal of empty columns is finite

ROW_TILED = os.environ.get("K_ROW_TILED", "0") == "1"

# Term order (magnitude tiers): lhsT row i pairs with rhs row i.
#   q_j = bf16 split j of depths^2   (rhs = ones)
#   d_j = bf16 split j of depths     (rhs = m_k = split k of -2*mu)
#   s_j = bf16 split j of mu^2+mask  (lhsT = ones)
_LHS_ROWS = ["q1", "d1", "1", "q2", "d1", "d2", "1", "q3", "d1", "d2", "d3", "1",
             "q4", "d2", "d3", "1"]
_RHS_ROWS = ["1", "m1", "s1", "1", "m2", "m1", "s2", "1", "m3", "m2", "m1", "s3",
             "1", "m3", "m2", "s4"]

_CACHE = {}


def _bf16_splits(x, n):
    out = []
    r = np.asarray(x, np.float32).copy()
    for _ in range(n):
        h = r.astype(ml_dtypes.bfloat16)
        out.append(h)
        r = r - h.astype(np.float32)
    return out


def _constants():
    import jax.numpy as jnp

    angles = np.asarray(jnp.linspace(-30.0, 30.0, A), dtype=np.float32)  # degrees
    depths = np.asarray(jnp.linspace(0.0, MAX_DEPTH, D), dtype=np.float32)
    angles_bcast = np.ascontiguousarray(np.broadcast_to(angles, (ROWS, A)))

    d1, d2, d3 = _bf16_splits(depths, 3)
    q1, q2, q3, q4 = _bf16_splits(depths * depths, 4)
    ones = np.ones(D, ml_dtypes.bfloat16)
    terms = {"d1": d1, "d2": d2, "d3": d3, "q1": q1, "q2": q2, "q3": q3, "q4": q4,
             "1": ones}

    if ROW_TILED:
        lhsT = np.zeros((128, 128), ml_dtypes.bfloat16)
        for i in range(DBLK):
            for j, nm in enumerate(_LHS_ROWS):
                lhsT[32 * i + j] = terms[nm][i * 128 : (i + 1) * 128]
    else:
        lhsT = np.zeros((K16, D), ml_dtypes.bfloat16)
        for j, nm in enumerate(_LHS_ROWS):
            lhsT[j] = terms[nm]
    return angles_bcast, np.ascontiguousarray(lhsT)


def _build():
    nc = bacc.Bacc("TRN2", target_bir_lowering=False, debug=False, num_devices=N_CORES)

    lhsT_shape = (128, 128) if ROW_TILED else (K16, D)
    p_loc_d = nc.dram_tensor("p_loc", (ROWS, 3), F32, kind="ExternalInput").ap()
    ang_d = nc.dram_tensor("angles", (ROWS, A), F32, kind="ExternalInput").ap()
    lhsT_d = nc.dram_tensor("lhsT", lhsT_shape, BF16, kind="ExternalInput").ap()
    rones_d = nc.dram_tensor("rhs_ones", (1, ROWS * A), BF16, kind="ExternalInput").ap()
    onesc_d = nc.dram_tensor(
        "ones_col", (128, 128), F32R, kind="ExternalInput"
    ).ap()
    onesr_d = nc.dram_tensor(
        "ones_eps", (1, 128 + W), F32R, kind="ExternalInput"
    ).ap()
    out_d = nc.dram_tensor("out", (ROWS, D, A), F32, kind="ExternalOutput").ap()

    with tile.TileContext(nc) as tc, ExitStack() as ctx:
        const = ctx.enter_context(tc.tile_pool(name="const", bufs=1))
        prep = ctx.enter_context(tc.tile_pool(name="prep", bufs=1))
        pun_pool = ctx.enter_context(tc.tile_pool(name="pun", bufs=4))
        pout_pool = ctx.enter_context(tc.tile_pool(name="pout", bufs=6))
        rec_pool = ctx.enter_context(tc.tile_pool(name="rec", bufs=4))
        sq_pool = ctx.enter_context(tc.tile_pool(name="sq", bufs=3, space="PSUM"))
        fb_pool = ctx.enter_context(tc.tile_pool(name="fb", bufs=2, space="PSUM"))

        # ---- constants / inputs ----
        p_sb = const.tile([ROWS, 3], F32)
        ang_sb = const.tile([ROWS, A], F32)
        lhsT_sb = const.tile(list(lhsT_shape), BF16)
        nc.sync.dma_start(p_sb, p_loc_d)
        nc.sync.dma_start(ang_sb, ang_d)
        nc.sync.dma_start(lhsT_sb, lhsT_d)

        ones_col = const.tile([128, 128], F32R)  # colsum weights (K=128, M=128)
        nc.sync.dma_start(ones_col, onesc_d)
        ones_eps = const.tile([1, 128 + W], F32R)
        nc.sync.dma_start(ones_eps, onesr_d)
        ones_row = ones_eps[:, 0:128]  # eps weights (K=1, M=128)
        eps_row = ones_eps[:, 128 : 128 + W]

        # ---- per-row prep: mu, masked rhs rows ----
        theta_bias = prep.tile([ROWS, 1], F32)
        # theta_p * pi/180 + pi/2   (pi/2 turns ACT Sin into cos)
        nc.vector.tensor_scalar(
            theta_bias, p_sb[:, 2:3], DEG2RAD, float(np.pi / 2), ALU.mult, ALU.add
        )
        cosang = prep.tile([ROWS, A], F32)
        nc.scalar.activation(cosang, ang_sb, AF.Sin, bias=theta_bias[:], scale=DEG2RAD)
        rcos = prep.tile([ROWS, A], F32)
        scratch = prep.tile([ROWS, A], F32)
        nc.vector.reciprocal_approx_accurate(rcos, cosang, scratch)
        mu = prep.tile([ROWS, A], F32)
        nc.vector.tensor_scalar_mul(mu, rcos, p_sb[:, 1:2])
        m2mu = prep.tile([ROWS, A], F32)
        nc.vector.tensor_scalar_mul(m2mu, mu, -2.0)
        musq = prep.tile([ROWS, A], F32)
        nc.vector.tensor_mul(musq, mu, mu)
        v_lt = prep.tile([ROWS, A], F32)
        nc.vector.tensor_scalar(v_lt, mu, float(MAX_DEPTH), None, ALU.is_lt)
        v_gt = prep.tile([ROWS, A], F32)
        nc.vector.tensor_scalar(v_gt, mu, 0.0, None, ALU.is_gt)
        vld = prep.tile([ROWS, A], F32)
        nc.vector.tensor_mul(vld, v_lt, v_gt)

        m2mu_m = prep.tile([ROWS, A], F32)
        nc.vector.tensor_mul(m2mu_m, m2mu, vld)
        musq_v = prep.tile([ROWS, A], F32)
        nc.vector.tensor_mul(musq_v, musq, vld)
        bigt = prep.tile([ROWS, A], F32)
        nc.vector.tensor_scalar(bigt, vld, -BIG, BIG, ALU.mult, ALU.add)
        musq_m = prep.tile([ROWS, A], F32)
        nc.vector.tensor_add(musq_m, musq_v, bigt)

        # ---- bf16 splits of -2*mu (3 terms) and mu^2+mask (4 terms) ----
        def dev_splits(src, n, tag):
            outs = []
            r = src
            for t in range(n):
                h = prep.tile([ROWS, A], BF16, tag=f"{tag}h{t}")
                nc.vector.tensor_copy(h, r)
                outs.append(h)
                if t < n - 1:
                    nr = prep.tile([ROWS, A], F32, tag=f"{tag}r{t}")
                    nc.vector.tensor_tensor(nr, r, h, ALU.subtract)
                    r = nr
            return outs

        m1, m2, m3 = dev_splits(m2mu_m, 3, "m")
        s1, s2, s3, s4 = dev_splits(musq_m, 4, "s")
        rhs_srcs = {"m1": m1, "m2": m2, "m3": m3, "s1": s1, "s2": s2, "s3": s3,
                    "s4": s4}

        # ---- flatten rhs rows to [1, 16384] bf16 ----
        if ROW_TILED:
            rhs_all = const.tile([128, ROWS * A], BF16)
            bases = [32 * i for i in range(DBLK)]
        else:
            rhs_all = const.tile([K16, ROWS * A], BF16)
            bases = [0]
        for b0 in bases:
            for j, nm in enumerate(_RHS_ROWS):
                row = rhs_all[b0 + j : b0 + j + 1, :]
                if nm == "1":
                    nc.sync.dma_start(row, rones_d)
                else:
                    nc.sync.dma_start(row, rhs_srcs[nm][:, :])

        # ---- main loop over pairs ----
        for pl in range(PAIRS):
            sq_tiles = []
            for h in range(DBLK // 2):
                sq_ps = sq_pool.tile([128, 2 * W], F32, tag="sq")
                sq_tiles.append(sq_ps)
                for j in range(2):
                    k = 2 * h + j
                    if ROW_TILED:
                        nc.tensor.matmul(
                            sq_ps[:, j * W : (j + 1) * W],
                            lhsT_sb[32 * k : 32 * k + K16, :],
                            rhs_all[32 * k : 32 * k + K16, pl * W : (pl + 1) * W],
                            start=True,
                            stop=True,
                            tile_position=(32 * k, 0),
                        )
                    else:
                        nc.tensor.matmul(
                            sq_ps[:, j * W : (j + 1) * W],
                            lhsT_sb[:, k * 128 : (k + 1) * 128],
                            rhs_all[:, pl * W : (pl + 1) * W],
                            start=True,
                            stop=True,
                        )

            p_un = pun_pool.tile([128, DBLK * W], F32R)
            for h in range(DBLK // 2):
                nc.scalar.activation(
                    p_un[:, 2 * h * W : 2 * (h + 1) * W],
                    sq_tiles[h],
                    AF.Exp,
                    scale=EXP_SCALE,
                )

            fb_ps = fb_pool.tile([128, W], F32)
            if ROW_TILED:
                nc.tensor.matmul(
                    fb_ps, ones_row, eps_row, start=True, stop=False,
                    tile_position=(0, 0),
                )
                for k in range(DBLK):
                    for g in range(4):
                        nc.tensor.matmul(
                            fb_ps,
                            ones_col[32 * g : 32 * g + 32, :],
                            p_un[32 * g : 32 * g + 32, k * W : (k + 1) * W],
                            start=False,
                            stop=(k == DBLK - 1 and g == 3),
                            tile_position=(32 * g, 0),
                        )
            else:
                nc.tensor.matmul(fb_ps, ones_row, eps_row, start=True, stop=False)
                for k in range(DBLK):
                    nc.tensor.matmul(
                        fb_ps,
                        ones_col,
                        p_un[:, k * W : (k + 1) * W],
                        start=False,
                        stop=(k == DBLK - 1),
                    )

            rec = rec_pool.tile([128, W], F32)
            nc.vector.reciprocal_approx_fast(rec, fb_ps)

            # p_out free layout is (b, dblk, a) so the store below is a
            # single 3-dim AP on both sides (src fully contiguous).
            p_out = pout_pool.tile([128, DBLK * W], F32)
            p_out_r = p_out.rearrange("p (b k a) -> p b k a", b=2, k=DBLK)
            rec_r = rec.rearrange("p (b a) -> p b a", b=2)
            pun_r = p_un.bitcast(F32).rearrange("p (k b a) -> p b k a", k=DBLK, b=2)
            # DVE: depth blocks 0..2 in a single op; rec broadcast over the
            # k axis via a stride-0 AP dim.
            rec_k3 = bass.AP(
                tensor=rec.tensor,
                offset=rec.offset,
                ap=[rec.ap[0], [A, 2], [0, 3], [1, A]],
            )
            nc.vector.tensor_tensor(
                p_out_r[:, :, 0:3, :], pun_r[:, :, 0:3, :], rec_k3, ALU.mult
            )
            nc.gpsimd.tensor_mul(
                p_out_r[:, :, 3, :],
                p_un[:, 3 * W : 4 * W].bitcast(F32).rearrange("p (b a) -> p b a", b=2),
                rec_r,
            )

            # one contiguous 1 MiB store: rows (2*pl, 2*pl+1) of out
            dst = bass.AP(
                tensor=out_d.tensor,
                offset=(2 * pl) * D * A,
                ap=[[A, 128], [128 * A, 2 * DBLK], [1, A]],
            )
            nc.sync.dma_start(dst, p_out)

    nc.compile()
    return nc


def _get_nc():
    if "nc" not in _CACHE:
        _CACHE["nc"] = _build()
    return _CACHE["nc"]


def _in_maps(p, p_calib):
    angles_bcast, lhsT = _constants()

    ones_col_np = np.ones((128, 128), np.float32)
    ones_eps_np = np.concatenate(
        [np.ones(128, np.float32), np.full(W, EPS, np.float32)]
    )[None, :]
    rhs_ones_np = np.ones((1, ROWS * A), ml_dtypes.bfloat16)

    in_maps = []
    for c in range(N_CORES):
        p_loc = np.concatenate(
            [p[c * BLOC : (c + 1) * BLOC], p_calib[c * BLOC : (c + 1) * BLOC]], axis=0
        )
        in_maps.append(
            {
                "p_loc": np.ascontiguousarray(p_loc),
                "angles": angles_bcast,
                "lhsT": lhsT,
                "ones_col": ones_col_np,
                "ones_eps": np.ascontiguousarray(ones_eps_np),
                "rhs_ones": rhs_ones_np,
            }
        )
    return in_maps


def kernel(p, p_calib):
    p = np.ascontiguousarray(np.asarray(p, dtype=np.float32))
    p_calib = np.ascontiguousarray(np.asarray(p_calib, dtype=np.float32))
    assert p.shape == (B, 3) and p_calib.shape == (B, 3)

    nc = _get_nc()
    in_maps = _in_maps(p, p_calib)
    res = bass_utils.run_bass_kernel_spmd(nc, in_maps, core_ids=list(range(N_CORES)))

    P_d = np.empty((B, D, A), np.float32)
    P_c = np.empty((B, D, A), np.float32)
    for c in range(N_CORES):
        o = res.results[c]["out"]
        P_d[c * BLOC : (c + 1) * BLOC] = o[:BLOC]
        P_c[c * BLOC : (c + 1) * BLOC] = o[BLOC:]
    return (P_d, P_c)
